# revision 4
# baseline (speedup 1.0000x reference)
"""Multi-head attention Trainium2 kernel (B=4, S=2048, D=1024, H=16).

Sharding: tensor-parallel over heads — each of the 8 cores owns 2 heads
(128 of the 1024 hidden dims). Each core computes its heads' q/k/v
projections, full attention for those heads, and a partial output
projection (contraction over its 128 ctx dims). The 8 partial outputs are
summed on the host (out_linear row-parallel; host-side reduce instead of
an on-device all-reduce since the contract takes/returns full tensors).

Device layout choices:
 - All matmul operands are produced in the orientation the tensor engine
   wants, using host-pretransposed x^T and W^T, so no on-device PE
   transposes are needed (v is reoriented with a DMA transpose).
 - Softmax uses unnormalized exp (scores are O(1) here, exp can't
   overflow); the denominator is obtained for free by appending a ones
   column to v in the ctx matmul, and normalization is folded into the
   PSUM->SBUF evacuation of ctx^T.
 - bv and bo never enter the nonlinearity, so their contribution
   (Wo @ bv + bo) is added on the host.
"""

import numpy as np
import ml_dtypes

B, S, D, H = 4, 2048, 1024, 16
HD = 64          # head dim
NCORES = 8
OC = 128         # per-core slice of hidden dim (2 heads x 64)
DC = D // 128    # 8 contraction chunks for the projections
KC = S // 128    # 16 key chunks
QH = S // 512    # 4 query blocks of 512
BF16 = ml_dtypes.bfloat16

_CACHE = {}


def _build():
    import concourse.bass as bass  # noqa: F401
    import concourse.tile as tile
    from concourse import bacc, mybir

    bf16 = mybir.dt.bfloat16
    f32 = mybir.dt.float32
    Exp = mybir.ActivationFunctionType.Exp

    nc = bacc.Bacc(
        "TRN2",
        target_bir_lowering=False,
        debug=False,
        enable_asserts=False,
        num_devices=NCORES,
    )

    xT = nc.dram_tensor("xT", [B, D, S], bf16, kind="ExternalInput").ap()
    wqkv = nc.dram_tensor("wqkv", [DC, 128, 3 * OC], bf16, kind="ExternalInput").ap()
    wo = nc.dram_tensor("wo", [OC, D], bf16, kind="ExternalInput").ap()
    b2 = nc.dram_tensor("b2", [OC, 2], f32, kind="ExternalInput").ap()
    outp = nc.dram_tensor("outp", [B, D, S], f32, kind="ExternalOutput").ap()

    with tile.TileContext(nc) as tc:
        with (
            tc.tile_pool(name="const", bufs=1) as const_pool,
            tc.tile_pool(name="xb", bufs=2) as xb_pool,
            tc.tile_pool(name="qk", bufs=2) as qk_pool,
            tc.tile_pool(name="vts", bufs=2) as vts_pool,
            tc.tile_pool(name="vnat", bufs=2) as vnat_pool,
            tc.tile_pool(name="attn", bufs=3) as attn_pool,
            tc.tile_pool(name="ctxs", bufs=2) as ctxs_pool,
            tc.tile_pool(name="small", bufs=4) as small_pool,
            tc.tile_pool(name="ostage", bufs=4) as ostage_pool,
            tc.tile_pool(name="scores_p", bufs=1, space="PSUM") as scores_pool,
            tc.tile_pool(name="ctx_p", bufs=1, space="PSUM") as ctx_pool,
            tc.tile_pool(name="mm_p", bufs=2, space="PSUM") as mm_pool,
        ):
            # Constant weights, loaded once.
            wqkv_s = const_pool.tile([128, DC, 3 * OC], bf16, tag="wqkv")
            nc.sync.dma_start(wqkv_s[:], wqkv.rearrange("c p n -> p c n"))
            wo_s = const_pool.tile([OC, D], bf16, tag="wo")
            nc.sync.dma_start(wo_s[:], wo)
            b2_s = const_pool.tile([OC, 2], f32, tag="b2")
            nc.sync.dma_start(b2_s[:], b2)

            for b in range(B):
                # ---- projections: qT/kT = W @ xT (+bias), vT = Wv @ xT ----
                xb_t = xb_pool.tile([128, DC, S], bf16, tag="xb")
                xsrc = xT[b].rearrange("(c p) s -> p c s", p=128)
                nc.sync.dma_start(xb_t[:, 0 : DC // 2, :], xsrc[:, 0 : DC // 2, :])
                nc.sync.dma_start(xb_t[:, DC // 2 :, :], xsrc[:, DC // 2 :, :])

                qT_t = qk_pool.tile([128, S], bf16, tag="qT")
                kT_t = qk_pool.tile([128, S], bf16, tag="kT")
                vh = [
                    vts_pool.tile([64, S], bf16, tag=f"vh{h}", name=f"vh{h}_{b}")
                    for h in range(2)
                ]
                for ti in range(3):
                    for sc in range(4):
                        ps = mm_pool.tile([128, 512], f32, tag="mm")
                        for dc in range(DC):
                            nc.tensor.matmul(
                                ps[:],
                                lhsT=wqkv_s[:, dc, ti * OC : (ti + 1) * OC],
                                rhs=xb_t[:, dc, sc * 512 : (sc + 1) * 512],
                                start=(dc == 0),
                                stop=(dc == DC - 1),
                            )
                        if ti < 2:
                            dest = qT_t if ti == 0 else kT_t
                            nc.vector.tensor_scalar_add(
                                dest[:, sc * 512 : (sc + 1) * 512],
                                ps[:],
                                b2_s[:, ti : ti + 1],
                            )
                        else:
                            for h in range(2):
                                nc.vector.tensor_copy(
                                    vh[h][:, sc * 512 : (sc + 1) * 512],
                                    ps[h * 64 : (h + 1) * 64, :],
                                )

                # v in natural orientation [k, hd] per 128-key chunk, one
                # ones-column appended per head: [v_h0(64)|1|pad, v_h1(64)|1|pad].
                # DMA-transpose needs 16-element-aligned dest offsets and
                # partition-0-based sources, hence the 80-wide head stride.
                v_t = vnat_pool.tile([128, KC, 160], bf16, tag="vnat")
                nc.gpsimd.memset(v_t[:, :, 64:65], 1.0)
                nc.gpsimd.memset(v_t[:, :, 144:145], 1.0)
                for i in range(KC):
                    for h in range(2):
                        nc.sync.dma_start(
                            v_t[:, i, h * 80 : h * 80 + 64],
                            vh[h][:, i * 128 : (i + 1) * 128],
                            transpose=True,
                        )

                ctxT_t = ctxs_pool.tile([128, S], bf16, tag="ctxT")

                # ---- attention ----
                for qh in range(QH):
                    ctx_ps = [
                        ctx_pool.tile(
                            [65, 512], f32, tag=f"ctx_h{h}", name=f"ctx_h{h}_{b}_{qh}"
                        )
                        for h in range(2)
                    ]
                    for pair in range(KC // 2):
                        sc_t = scores_pool.tile([128, 2048], f32, tag="scores")
                        for t in range(2):
                            kc = 2 * pair + t
                            for h in range(2):
                                nc.tensor.matmul(
                                    sc_t[:, t * 1024 + h * 512 : t * 1024 + (h + 1) * 512],
                                    lhsT=kT_t[h * 64 : (h + 1) * 64, kc * 128 : (kc + 1) * 128],
                                    rhs=qT_t[h * 64 : (h + 1) * 64, qh * 512 : (qh + 1) * 512],
                                    start=True,
                                    stop=True,
                                )
                        at_t = attn_pool.tile([128, 2048], bf16, tag="attn")
                        nc.scalar.activation(at_t[:], sc_t[:], Exp, scale=0.125)
                        for t in range(2):
                            kc = 2 * pair + t
                            for h in range(2):
                                nc.tensor.matmul(
                                    ctx_ps[h][:],
                                    lhsT=v_t[:, kc, h * 80 : h * 80 + 65],
                                    rhs=at_t[:, t * 1024 + h * 512 : t * 1024 + (h + 1) * 512],
                                    start=(kc == 0),
                                    stop=(kc == KC - 1),
                                )
                    # normalize: ctxT row 64 is the softmax denominator
                    for h in range(2):
                        rc_t = small_pool.tile([1, 512], f32, tag="recip")
                        nc.vector.reciprocal(rc_t[:], ctx_ps[h][64:65, :])
                        bc_t = small_pool.tile([64, 512], f32, tag="bcast")
                        nc.gpsimd.partition_broadcast(bc_t[:], rc_t[:])
                        nc.vector.tensor_mul(
                            ctxT_t[h * 64 : (h + 1) * 64, qh * 512 : (qh + 1) * 512],
                            ctx_ps[h][0:64, :],
                            bc_t[:],
                        )

                # ---- output projection (partial over this core's 128 dims) ----
                for it in range(8):
                    for sc in range(4):
                        ps = mm_pool.tile([128, 512], f32, tag="mm")
                        nc.tensor.matmul(
                            ps[:],
                            lhsT=wo_s[:, it * 128 : (it + 1) * 128],
                            rhs=ctxT_t[:, sc * 512 : (sc + 1) * 512],
                            start=True,
                            stop=True,
                        )
                        ot = ostage_pool.tile([128, 512], f32, tag="ostage")
                        nc.vector.tensor_copy(ot[:], ps[:])
                        nc.sync.dma_start(
                            outp[b, it * 128 : (it + 1) * 128, sc * 512 : (sc + 1) * 512],
                            ot[:],
                        )

    nc.compile()
    return nc


def _prep_inputs(x, Wq, bq, Wk, bk, Wv, bv, Wo, bo):
    x = np.asarray(x, np.float32)
    xT = np.ascontiguousarray(x.transpose(0, 2, 1)).astype(BF16)
    in_maps = []
    for c in range(NCORES):
        sl = slice(c * OC, (c + 1) * OC)
        wqkv_c = np.concatenate(
            [np.asarray(W, np.float32)[sl, :].T for W in (Wq, Wk, Wv)], axis=1
        )  # [D, 384]
        wqkv_c = np.ascontiguousarray(wqkv_c.reshape(DC, 128, 3 * OC)).astype(BF16)
        wo_c = np.ascontiguousarray(np.asarray(Wo, np.float32)[:, sl].T).astype(BF16)
        b2_c = np.ascontiguousarray(
            np.stack([np.asarray(bq, np.float32)[sl], np.asarray(bk, np.float32)[sl]], axis=1)
        )
        in_maps.append({"xT": xT, "wqkv": wqkv_c, "wo": wo_c, "b2": b2_c})
    return in_maps


def kernel(x, Wq, bq, Wk, bk, Wv, bv, Wo, bo):
    from concourse import bass_utils

    if "nc" not in _CACHE:
        _CACHE["nc"] = _build()
    nc = _CACHE["nc"]

    in_maps = _prep_inputs(x, Wq, bq, Wk, bk, Wv, bv, Wo, bo)
    import os

    res = bass_utils.run_bass_kernel_spmd(
        nc,
        in_maps,
        core_ids=list(range(NCORES)),
        trace=bool(os.environ.get("BASS_TRACE")),
    )
    _CACHE["last_results"] = res

    total = np.zeros((B, D, S), np.float32)
    for r in res.results:
        total += r["outp"]
    out = total.transpose(0, 2, 1)
    out = out + (np.asarray(Wo, np.float32) @ np.asarray(bv, np.float32))[None, None, :]
    out = out + np.asarray(bo, np.float32)[None, None, :]
    return np.ascontiguousarray(out.astype(np.float32))


# revision 10
# speedup vs baseline: 6.3468x; 6.3468x over previous
"""Multi-head attention Trainium2 kernel (B=4, S=2048, D=1024, H=16).

Sharding: tensor-parallel over heads — each of the 8 cores owns 2 heads
(128 of the 1024 hidden dims). Each core computes its heads' q/k/v
projections, full attention for those heads, and a partial output
projection (contraction over its 128 ctx dims). The 8 partial outputs are
summed on the host (out_linear row-parallel; host-side reduce instead of
an on-device all-reduce since the contract takes/returns full tensors).

Device layout choices:
 - All matmul operands are produced in the orientation the tensor engine
   wants, using host-pretransposed x^T and W^T, so no on-device PE
   transposes are needed (v is reoriented with a DMA transpose).
 - Softmax uses unnormalized exp (scores are O(1) here, exp can't
   overflow); the denominator is obtained for free by appending a ones
   column to v in the ctx matmul, and normalization is folded into the
   PSUM->SBUF evacuation of ctx^T.
 - bv and bo never enter the nonlinearity, so their contribution
   (Wo @ bv + bo) is added on the host.
"""

import numpy as np
import ml_dtypes

B, S, D, H = 4, 2048, 1024, 16
HD = 64          # head dim
NCORES = 8
OC = 128         # per-core slice of hidden dim (2 heads x 64)
DC = D // 128    # 8 contraction chunks for the projections
KC = S // 128    # 16 key chunks
QH = S // 512    # 4 query blocks of 512
BF16 = ml_dtypes.bfloat16

_CACHE = {}


def _build(reps=1, hw_loop=0):
    import contextlib

    import concourse.bass as bass  # noqa: F401
    import concourse.tile as tile
    from concourse import bacc, mybir

    bf16 = mybir.dt.bfloat16
    f32 = mybir.dt.float32
    Exp = mybir.ActivationFunctionType.Exp

    nc = bacc.Bacc(
        "TRN2",
        target_bir_lowering=False,
        debug=False,
        enable_asserts=False,
        num_devices=NCORES,
    )

    xT = nc.dram_tensor("xT", [B, D, S], bf16, kind="ExternalInput").ap()
    wqkv = nc.dram_tensor("wqkv", [DC, 128, 3 * OC], bf16, kind="ExternalInput").ap()
    wo = nc.dram_tensor("wo", [OC, D], bf16, kind="ExternalInput").ap()
    b2 = nc.dram_tensor("b2", [OC, 2], f32, kind="ExternalInput").ap()
    outp = nc.dram_tensor("outp", [B, D, S], f32, kind="ExternalOutput").ap()

    with tile.TileContext(nc) as tc:
        with (
            tc.tile_pool(name="const", bufs=1) as const_pool,
            tc.tile_pool(name="xb", bufs=2) as xb_pool,
            tc.tile_pool(name="qk", bufs=2) as qk_pool,
            tc.tile_pool(name="vts", bufs=2) as vts_pool,
            tc.tile_pool(name="vnat", bufs=2) as vnat_pool,
            tc.tile_pool(name="attn", bufs=3) as attn_pool,
            tc.tile_pool(name="ctxs", bufs=2) as ctxs_pool,
            tc.tile_pool(name="small", bufs=4) as small_pool,
            tc.tile_pool(name="ostage", bufs=4) as ostage_pool,
            tc.tile_pool(name="scores_p", bufs=1, space="PSUM") as scores_pool,
            tc.tile_pool(name="ctx_p", bufs=1, space="PSUM") as ctx_pool,
            tc.tile_pool(name="mm_p", bufs=2, space="PSUM") as mm_pool,
        ):
            # Constant weights, loaded once.
            wqkv_s = const_pool.tile([128, DC, 3 * OC], bf16, tag="wqkv")
            nc.sync.dma_start(wqkv_s[:], wqkv.rearrange("c p n -> p c n"))
            wo_s = const_pool.tile([OC, D], bf16, tag="wo")
            nc.sync.dma_start(wo_s[:], wo)
            b2_s = const_pool.tile([OC, 2], f32, tag="b2")
            nc.sync.dma_start(b2_s[:], b2)

            loop_cm = (
                tc.For_i(0, hw_loop, 1, name="reploop")
                if hw_loop
                else contextlib.nullcontext()
            )
            with loop_cm:
             for _rep in range(reps):
              for b in range(B):
                # ---- projections: qT/kT = W @ xT (+bias), vT = Wv @ xT ----
                xb_t = xb_pool.tile([128, DC, S], bf16, tag="xb")
                xsrc = xT[b].rearrange("(c p) s -> p c s", p=128)
                nc.sync.dma_start(xb_t[:, 0 : DC // 2, :], xsrc[:, 0 : DC // 2, :])
                nc.sync.dma_start(xb_t[:, DC // 2 :, :], xsrc[:, DC // 2 :, :])

                qT_t = qk_pool.tile([128, S], bf16, tag="qT")
                kT_t = qk_pool.tile([128, S], bf16, tag="kT")
                vh = [
                    vts_pool.tile([64, S], bf16, tag=f"vh{h}", name=f"vh{h}_{b}_r{_rep}")
                    for h in range(2)
                ]
                for ti in range(3):
                    for sc in range(4):
                        ps = mm_pool.tile([128, 512], f32, tag="mm")
                        for dc in range(DC):
                            nc.tensor.matmul(
                                ps[:],
                                lhsT=wqkv_s[:, dc, ti * OC : (ti + 1) * OC],
                                rhs=xb_t[:, dc, sc * 512 : (sc + 1) * 512],
                                start=(dc == 0),
                                stop=(dc == DC - 1),
                            )
                        if ti < 2:
                            dest = qT_t if ti == 0 else kT_t
                            nc.vector.tensor_scalar_add(
                                dest[:, sc * 512 : (sc + 1) * 512],
                                ps[:],
                                b2_s[:, ti : ti + 1],
                            )
                        else:
                            for h in range(2):
                                nc.vector.tensor_copy(
                                    vh[h][:, sc * 512 : (sc + 1) * 512],
                                    ps[h * 64 : (h + 1) * 64, :],
                                )

                # v in natural orientation [k, hd] per 128-key chunk, one
                # ones-column appended per head: [v_h0(64)|1|pad, v_h1(64)|1|pad].
                # DMA-transpose needs 16-element-aligned dest offsets and
                # partition-0-based sources, hence the 80-wide head stride.
                v_t = vnat_pool.tile([128, KC, 160], bf16, tag="vnat")
                nc.gpsimd.memset(v_t[:, :, 64:65], 1.0)
                nc.gpsimd.memset(v_t[:, :, 144:145], 1.0)
                for i in range(KC):
                    for h in range(2):
                        nc.sync.dma_start(
                            v_t[:, i, h * 80 : h * 80 + 64],
                            vh[h][:, i * 128 : (i + 1) * 128],
                            transpose=True,
                        )

                ctxT_t = ctxs_pool.tile([128, S], bf16, tag="ctxT")

                # ---- attention ----
                for qh in range(QH):
                    ctx_ps = [
                        ctx_pool.tile(
                            [65, 512], f32, tag=f"ctx_h{h}", name=f"ctx_h{h}_{b}_{qh}_r{_rep}"
                        )
                        for h in range(2)
                    ]
                    for pair in range(KC // 2):
                        sc_t = scores_pool.tile([128, 2048], f32, tag="scores")
                        for t in range(2):
                            kc = 2 * pair + t
                            for h in range(2):
                                nc.tensor.matmul(
                                    sc_t[:, t * 1024 + h * 512 : t * 1024 + (h + 1) * 512],
                                    lhsT=kT_t[h * 64 : (h + 1) * 64, kc * 128 : (kc + 1) * 128],
                                    rhs=qT_t[h * 64 : (h + 1) * 64, qh * 512 : (qh + 1) * 512],
                                    start=True,
                                    stop=True,
                                )
                        at_t = attn_pool.tile([128, 2048], bf16, tag="attn")
                        nc.scalar.activation(at_t[:], sc_t[:], Exp, scale=0.125)
                        for t in range(2):
                            kc = 2 * pair + t
                            for h in range(2):
                                nc.tensor.matmul(
                                    ctx_ps[h][:],
                                    lhsT=v_t[:, kc, h * 80 : h * 80 + 65],
                                    rhs=at_t[:, t * 1024 + h * 512 : t * 1024 + (h + 1) * 512],
                                    start=(kc == 0),
                                    stop=(kc == KC - 1),
                                )
                    # normalize: ctxT row 64 is the softmax denominator
                    for h in range(2):
                        rc_t = small_pool.tile([1, 512], f32, tag="recip")
                        nc.vector.reciprocal(rc_t[:], ctx_ps[h][64:65, :])
                        bc_t = small_pool.tile([64, 512], f32, tag="bcast")
                        nc.gpsimd.partition_broadcast(bc_t[:], rc_t[:])
                        nc.vector.tensor_mul(
                            ctxT_t[h * 64 : (h + 1) * 64, qh * 512 : (qh + 1) * 512],
                            ctx_ps[h][0:64, :],
                            bc_t[:],
                        )

                # ---- output projection (partial over this core's 128 dims) ----
                for it in range(8):
                    for sc in range(4):
                        ps = mm_pool.tile([128, 512], f32, tag="mm")
                        nc.tensor.matmul(
                            ps[:],
                            lhsT=wo_s[:, it * 128 : (it + 1) * 128],
                            rhs=ctxT_t[:, sc * 512 : (sc + 1) * 512],
                            start=True,
                            stop=True,
                        )
                        ot = ostage_pool.tile([128, 512], f32, tag="ostage")
                        nc.vector.tensor_copy(ot[:], ps[:])
                        nc.sync.dma_start(
                            outp[b, it * 128 : (it + 1) * 128, sc * 512 : (sc + 1) * 512],
                            ot[:],
                        )

    nc.compile()
    return nc


def _prep_inputs(x, Wq, bq, Wk, bk, Wv, bv, Wo, bo):
    x = np.asarray(x, np.float32)
    xT = np.ascontiguousarray(x.transpose(0, 2, 1)).astype(BF16)
    in_maps = []
    for c in range(NCORES):
        sl = slice(c * OC, (c + 1) * OC)
        wqkv_c = np.concatenate(
            [np.asarray(W, np.float32)[sl, :].T for W in (Wq, Wk, Wv)], axis=1
        )  # [D, 384]
        wqkv_c = np.ascontiguousarray(wqkv_c.reshape(DC, 128, 3 * OC)).astype(BF16)
        wo_c = np.ascontiguousarray(np.asarray(Wo, np.float32)[:, sl].T).astype(BF16)
        b2_c = np.ascontiguousarray(
            np.stack([np.asarray(bq, np.float32)[sl], np.asarray(bk, np.float32)[sl]], axis=1)
        )
        in_maps.append({"xT": xT, "wqkv": wqkv_c, "wo": wo_c, "b2": b2_c})
    return in_maps


def kernel(x, Wq, bq, Wk, bk, Wv, bv, Wo, bo):
    from concourse import bass_utils

    if "nc" not in _CACHE:
        _CACHE["nc"] = _build()
    nc = _CACHE["nc"]

    in_maps = _prep_inputs(x, Wq, bq, Wk, bk, Wv, bv, Wo, bo)
    import os

    res = bass_utils.run_bass_kernel_spmd(
        nc,
        in_maps,
        core_ids=list(range(NCORES)),
        trace=bool(os.environ.get("BASS_TRACE")),
    )
    _CACHE["last_results"] = res

    total = np.zeros((B, D, S), np.float32)
    for r in res.results:
        total += r["outp"]
    out = total.transpose(0, 2, 1)
    out = out + (np.asarray(Wo, np.float32) @ np.asarray(bv, np.float32))[None, None, :]
    out = out + np.asarray(bo, np.float32)[None, None, :]
    return np.ascontiguousarray(out.astype(np.float32))


# revision 13
# speedup vs baseline: 9.4119x; 1.4829x over previous
"""Multi-head attention Trainium2 kernel (B=4, S=2048, D=1024, H=16).

Sharding: tensor-parallel over heads — each of the 8 cores owns 2 heads
(128 of the 1024 hidden dims). Each core computes its heads' q/k/v
projections, full attention for those heads, and a partial output
projection (contraction over its 128 ctx dims). The 8 partial outputs are
summed on the host (out_linear row-parallel; host-side reduce instead of
an on-device all-reduce since the contract takes/returns full tensors).

Device layout choices:
 - All matmul operands are produced in the orientation the tensor engine
   wants, using host-pretransposed x^T and W^T, so no on-device PE
   transposes are needed (v is reoriented with a DMA transpose).
 - Softmax uses unnormalized exp (scores are O(1) here, exp can't
   overflow); the denominator is obtained for free by appending a ones
   column to v in the ctx matmul, and normalization is folded into the
   PSUM->SBUF evacuation of ctx^T.
 - bv and bo never enter the nonlinearity, so their contribution
   (Wo @ bv + bo) is added on the host.
"""

import numpy as np
import ml_dtypes

B, S, D, H = 4, 2048, 1024, 16
HD = 64          # head dim
NCORES = 8
OC = 128         # per-core slice of hidden dim (2 heads x 64)
DC = D // 128    # 8 contraction chunks for the projections
KC = S // 128    # 16 key chunks
QH = S // 512    # 4 query blocks of 512
BF16 = ml_dtypes.bfloat16

_CACHE = {}


def _build(reps=1, hw_loop=0):
    import contextlib

    import concourse.bass as bass  # noqa: F401
    import concourse.tile as tile
    from concourse import bacc, mybir

    bf16 = mybir.dt.bfloat16
    f32 = mybir.dt.float32
    Exp = mybir.ActivationFunctionType.Exp

    nc = bacc.Bacc(
        "TRN2",
        target_bir_lowering=False,
        debug=False,
        enable_asserts=False,
        num_devices=NCORES,
    )

    xT = nc.dram_tensor("xT", [B, D, S], bf16, kind="ExternalInput").ap()
    wqkv = nc.dram_tensor("wqkv", [DC, 128, 3 * OC], bf16, kind="ExternalInput").ap()
    wo = nc.dram_tensor("wo", [OC, D], bf16, kind="ExternalInput").ap()
    b2 = nc.dram_tensor("b2", [OC, 2], f32, kind="ExternalInput").ap()
    outp = nc.dram_tensor("outp", [B, D, S], f32, kind="ExternalOutput").ap()

    with tile.TileContext(nc) as tc:
        with (
            tc.tile_pool(name="const", bufs=1) as const_pool,
            tc.tile_pool(name="xb", bufs=2) as xb_pool,
            tc.tile_pool(name="qk", bufs=2) as qk_pool,
            tc.tile_pool(name="vts", bufs=2) as vts_pool,
            tc.tile_pool(name="vnat", bufs=2) as vnat_pool,
            tc.tile_pool(name="attn", bufs=4) as attn_pool,
            tc.tile_pool(name="ctxs", bufs=2) as ctxs_pool,
            tc.tile_pool(name="small", bufs=4) as small_pool,
            tc.tile_pool(name="ostage", bufs=4) as ostage_pool,
            tc.tile_pool(name="scores_p", bufs=2, space="PSUM") as scores_pool,
            tc.tile_pool(name="ctx_p", bufs=1, space="PSUM") as ctx_pool,
            tc.tile_pool(name="mm_p", bufs=2, space="PSUM") as mm_pool,
        ):
            # Constant weights, loaded once.
            wqkv_s = const_pool.tile([128, DC, 3 * OC], bf16, tag="wqkv")
            nc.sync.dma_start(wqkv_s[:], wqkv.rearrange("c p n -> p c n"))
            wo_s = const_pool.tile([OC, D], bf16, tag="wo")
            nc.sync.dma_start(wo_s[:], wo)
            b2_s = const_pool.tile([OC, 2], f32, tag="b2")
            nc.sync.dma_start(b2_s[:], b2)

            loop_cm = (
                tc.For_i(0, hw_loop, 1, name="reploop")
                if hw_loop
                else contextlib.nullcontext()
            )
            with loop_cm:
             for _rep in range(reps):
              for b in range(B):
                # ---- projections: qT/kT = W @ xT (+bias), vT = Wv @ xT ----
                xb_t = xb_pool.tile([128, DC, S], bf16, tag="xb")
                xsrc = xT[b].rearrange("(c p) s -> p c s", p=128)
                nc.sync.dma_start(xb_t[:, 0 : DC // 2, :], xsrc[:, 0 : DC // 2, :])
                nc.sync.dma_start(xb_t[:, DC // 2 :, :], xsrc[:, DC // 2 :, :])

                qT_t = qk_pool.tile([128, S], bf16, tag="qT")
                kT_t = qk_pool.tile([128, S], bf16, tag="kT")
                vh = [
                    vts_pool.tile([64, S], bf16, tag=f"vh{h}", name=f"vh{h}_{b}_r{_rep}")
                    for h in range(2)
                ]
                for ti in range(3):
                    for sc in range(4):
                        ps = mm_pool.tile([128, 512], f32, tag="mm")
                        for dc in range(DC):
                            nc.tensor.matmul(
                                ps[:],
                                lhsT=wqkv_s[:, dc, ti * OC : (ti + 1) * OC],
                                rhs=xb_t[:, dc, sc * 512 : (sc + 1) * 512],
                                start=(dc == 0),
                                stop=(dc == DC - 1),
                            )
                        if ti < 2:
                            dest = qT_t if ti == 0 else kT_t
                            nc.vector.tensor_scalar_add(
                                dest[:, sc * 512 : (sc + 1) * 512],
                                ps[:],
                                b2_s[:, ti : ti + 1],
                            )
                        else:
                            for h in range(2):
                                nc.vector.tensor_copy(
                                    vh[h][:, sc * 512 : (sc + 1) * 512],
                                    ps[h * 64 : (h + 1) * 64, :],
                                )

                # v in natural orientation [k, hd] per 128-key chunk, one
                # ones-column appended per head: [v_h0(64)|1|pad, v_h1(64)|1|pad].
                # DMA-transpose needs 16-element-aligned dest offsets and
                # partition-0-based sources, hence the 80-wide head stride.
                v_t = vnat_pool.tile([128, KC, 160], bf16, tag="vnat")
                nc.gpsimd.memset(v_t[:, :, 64:65], 1.0)
                nc.gpsimd.memset(v_t[:, :, 144:145], 1.0)
                for i in range(KC):
                    for h in range(2):
                        nc.sync.dma_start(
                            v_t[:, i, h * 80 : h * 80 + 64],
                            vh[h][:, i * 128 : (i + 1) * 128],
                            transpose=True,
                        )

                ctxT_t = ctxs_pool.tile([128, S], bf16, tag="ctxT")

                # ---- attention ----
                for qh in range(QH):
                    ctx_ps = [
                        ctx_pool.tile(
                            [65, 512], f32, tag=f"ctx_h{h}", name=f"ctx_h{h}_{b}_{qh}_r{_rep}"
                        )
                        for h in range(2)
                    ]
                    for kc in range(KC):
                        sc_t = scores_pool.tile([128, 1024], f32, tag="scores")
                        for h in range(2):
                            nc.tensor.matmul(
                                sc_t[:, h * 512 : (h + 1) * 512],
                                lhsT=kT_t[h * 64 : (h + 1) * 64, kc * 128 : (kc + 1) * 128],
                                rhs=qT_t[h * 64 : (h + 1) * 64, qh * 512 : (qh + 1) * 512],
                                start=True,
                                stop=True,
                            )
                        at_t = attn_pool.tile([128, 1024], bf16, tag="attn")
                        nc.scalar.activation(at_t[:], sc_t[:], Exp, scale=0.125)
                        for h in range(2):
                            nc.tensor.matmul(
                                ctx_ps[h][:],
                                lhsT=v_t[:, kc, h * 80 : h * 80 + 65],
                                rhs=at_t[:, h * 512 : (h + 1) * 512],
                                start=(kc == 0),
                                stop=(kc == KC - 1),
                            )
                    # normalize: ctxT row 64 is the softmax denominator
                    for h in range(2):
                        rc_t = small_pool.tile([1, 512], f32, tag="recip")
                        nc.vector.reciprocal(rc_t[:], ctx_ps[h][64:65, :])
                        bc_t = small_pool.tile([64, 512], f32, tag="bcast")
                        nc.gpsimd.partition_broadcast(bc_t[:], rc_t[:])
                        nc.vector.tensor_mul(
                            ctxT_t[h * 64 : (h + 1) * 64, qh * 512 : (qh + 1) * 512],
                            ctx_ps[h][0:64, :],
                            bc_t[:],
                        )

                # ---- output projection (partial over this core's 128 dims) ----
                for it in range(8):
                    for sc in range(4):
                        ps = mm_pool.tile([128, 512], f32, tag="mm")
                        nc.tensor.matmul(
                            ps[:],
                            lhsT=wo_s[:, it * 128 : (it + 1) * 128],
                            rhs=ctxT_t[:, sc * 512 : (sc + 1) * 512],
                            start=True,
                            stop=True,
                        )
                        ot = ostage_pool.tile([128, 512], f32, tag="ostage")
                        nc.vector.tensor_copy(ot[:], ps[:])
                        nc.sync.dma_start(
                            outp[b, it * 128 : (it + 1) * 128, sc * 512 : (sc + 1) * 512],
                            ot[:],
                        )

    nc.compile()
    return nc


def _prep_inputs(x, Wq, bq, Wk, bk, Wv, bv, Wo, bo):
    x = np.asarray(x, np.float32)
    xT = np.ascontiguousarray(x.transpose(0, 2, 1)).astype(BF16)
    in_maps = []
    for c in range(NCORES):
        sl = slice(c * OC, (c + 1) * OC)
        wqkv_c = np.concatenate(
            [np.asarray(W, np.float32)[sl, :].T for W in (Wq, Wk, Wv)], axis=1
        )  # [D, 384]
        wqkv_c = np.ascontiguousarray(wqkv_c.reshape(DC, 128, 3 * OC)).astype(BF16)
        wo_c = np.ascontiguousarray(np.asarray(Wo, np.float32)[:, sl].T).astype(BF16)
        b2_c = np.ascontiguousarray(
            np.stack([np.asarray(bq, np.float32)[sl], np.asarray(bk, np.float32)[sl]], axis=1)
        )
        in_maps.append({"xT": xT, "wqkv": wqkv_c, "wo": wo_c, "b2": b2_c})
    return in_maps


def kernel(x, Wq, bq, Wk, bk, Wv, bv, Wo, bo):
    from concourse import bass_utils

    if "nc" not in _CACHE:
        _CACHE["nc"] = _build()
    nc = _CACHE["nc"]

    in_maps = _prep_inputs(x, Wq, bq, Wk, bk, Wv, bv, Wo, bo)
    import os

    res = bass_utils.run_bass_kernel_spmd(
        nc,
        in_maps,
        core_ids=list(range(NCORES)),
        trace=bool(os.environ.get("BASS_TRACE")),
    )
    _CACHE["last_results"] = res

    total = np.zeros((B, D, S), np.float32)
    for r in res.results:
        total += r["outp"]
    out = total.transpose(0, 2, 1)
    out = out + (np.asarray(Wo, np.float32) @ np.asarray(bv, np.float32))[None, None, :]
    out = out + np.asarray(bo, np.float32)[None, None, :]
    return np.ascontiguousarray(out.astype(np.float32))
